# revision 132
# baseline (speedup 1.0000x reference)
"""SAGAN-style self-attention block on 8 trn2 NeuronCores.

Full inputs: x [8, 512, 64, 64], w_theta [64, 512], w_phi [64, 512],
w_g [256, 512], w_o [512, 256], gamma scalar.

Sharding: data-parallel over batch — one batch item per core. Each core runs
an identical Bass program over its own x[b]; weights are replicated.

Per-core math (C=512, n=H*W=4096, m=n/4=1024):
  theta = w_theta @ x            [64, 4096]
  phi   = pool2(w_phi @ x)       [64, 1024]
  g     = pool2(w_g @ x)         [256, 1024]
  S^T   = phi^T @ theta          [1024, 4096]   (scores, transposed layout)
  E     = exp(S^T)               (no max-subtraction needed: |S| < ~50)
  Z     = ones^T @ (tree-sum E)  [*, 4096]      (row sums, broadcast layout)
  att   = (g @ E) / Z            [256, 4096]
  out   = (gamma*w_o) @ att + x  [512, 4096]

Projections and scores run as float32r (full-rate fp32 on the PE at
free>=256); the projection weights arrive pre-quantized to bf16 (halving
their serial-DMA slot ahead of the x stream) and are upconverted on the
idle early DVE. The attend path (E, the exp-sum tree, g^T, att, w_o) runs
in bf16: same PE rate, half the SBUF, and 2x-rate DVE adds. The residual
add uses f32r x and fp32 psum; the output is stored as bf16 and upcast on
the host. Total relative error ~2.8e-3 against the fp32 reference, well
inside the 2e-2 budget.

Schedule: phase 1 is paced by the serial x DMA stream (the cost model
serializes all DMA traffic at ~360GB/s), so slice-0/1 scores ride in its PE
slack. Phase 2 pipelines attend(i) | scores(i+2) | exp-tree(i+2) per
iteration: ct-major ap accumulation with the Z matmul + reciprocal slotted
between the blocks, the tree split Pool/DVE one-plus slices ahead, and
residual outputs written with one batched DMA per chunk
([128, 4, w] -> four 128-row DRAM blocks, bf16). The last slice runs its
out-projection in two half chunks on alternating psum tags and DMA queues
so the exposed tail is short.
"""

import time
from contextlib import ExitStack

import ml_dtypes
import numpy as np

import bass_rust
import concourse.bass as bass
import concourse.mybir as mybir
import concourse.tile as tile
from concourse.bass_utils import run_bass_kernel_spmd
from concourse.masks import make_identity

P = 128
C = 512  # channels
C8 = 64  # theta/phi channels
C2 = 256  # g channels
N = 4096  # H*W
M = 1024  # pooled spatial
NS = 8  # n-slices
SL = 512  # n-slice width
MT = 8  # m-tiles of 128
F32 = mybir.dt.float32
F32R = mybir.dt.float32r
BF16 = mybir.dt.bfloat16
AX = mybir.AxisListType
ALU = mybir.AluOpType
ACTF = mybir.ActivationFunctionType


def _pool_view(ap):
    """[p, 512] slice of the conv output -> 5D maxpool view [p, h2, w2, dy, dx].

    Within an n-slice of 512 = 8 image rows: local n = (2*h2+dy)*64 + 2*w2+dx.
    """
    return ap.rearrange("p (h2 dy w2 dx) -> p h2 w2 dy dx", h2=4, dy=2, w2=32, dx=2)


def emit(nc, tc, ctx):
    x_f = nc.dram_tensor("x", [C, N], F32R, kind="ExternalInput")
    wproj = nc.dram_tensor("wproj", [C, 384], BF16, kind="ExternalInput")
    wo = nc.dram_tensor("wo", [C2, C], BF16, kind="ExternalInput")
    out_d = nc.dram_tensor("out", [C, N], BF16, kind="ExternalOutput")
    out_v = out_d.ap().rearrange("(ot p) n -> p ot n", ot=4)

    persist = ctx.enter_context(tc.tile_pool(name="persist", bufs=1))

    # wproj arrives pre-quantized to bf16 (half the serial-DMA slot ahead
    # of the x stream) and upconverts to f32r on the otherwise-idle early DVE
    wpb = persist.tile([P, 4, 384], BF16, name="wpb")
    wpt = persist.tile([P, 4, 384], F32R, name="wpt")
    for k in range(4):
        nc.scalar.dma_start(out=wpb[:, k, :], in_=wproj.ap()[k * P : (k + 1) * P, :])
        nc.vector.tensor_copy(wpt[:, k, :], wpb[:, k, :])
    wp = [wpt[:, k, :] for k in range(4)]
    ones_f = persist.tile([P, P], F32)
    nc.vector.memset(ones_f, 1.0)
    ones_b = persist.tile([P, P], BF16)
    nc.vector.tensor_copy(ones_b, ones_f)
    ident_f = persist.tile([P, P], F32)
    make_identity(nc, ident_f)
    ident = persist.tile([P, P], F32R)
    nc.vector.tensor_copy(ident, ident_f)

    # score psum pool lives across phases 1+2 so slice-0 can score inside
    # phase 1
    spool = ctx.enter_context(tc.tile_pool(name="spsum", bufs=2, space="PSUM"))
    etp = ctx.enter_context(tc.tile_pool(name="et", bufs=3))
    miscp = ctx.enter_context(tc.tile_pool(name="misc", bufs=2))

    # Startup: dummy exp preloads the ACT exp table (real-hw concern only);
    # the warmup matmuls start the PE p-state ramp clock early (full speed
    # ~3us after the first PE instruction) and keep the PE busy while the
    # first x/w DMAs land.
    actwarm = persist.tile([P, 1], F32)
    nc.scalar.activation(actwarm, ones_f[:, 0:1], ACTF.Exp)
    for wi in range(7):
        wt_ = spool.tile([P, P], F32, name="warm", tag=f"s{wi % 4}", bufs=1)
        nc.tensor.matmul(wt_, lhsT=ones_f, rhs=ones_f, start=True, stop=True)

    # x loads: slice-major chunks so phase-1 slice 0 unblocks after ~1MB.
    # Tiles are f32r (rounded at DMA time): they feed the projection matmuls
    # directly and the residual adds read them back via bitcast — the ~1e-4
    # relative rounding on the residual is well inside the error budget.
    xfa = persist.tile([P, 4, N], F32R, name="xfa")
    for q in range(NS):
        for cc in range(4):
            nc.sync.dma_start(
                out=xfa[:, cc, q * SL : (q + 1) * SL],
                in_=x_f[cc * P : (cc + 1) * P, q * SL : (q + 1) * SL],
            )
    # wot loads go after the x stream: they are not needed until the first
    # out-projection (~34us), and ahead of x they would delay phase 1
    wot = []
    for k in range(2):
        t = persist.tile([P, C], BF16, name=f"wot{k}")
        nc.sync.dma_start(out=t, in_=wo[k * P : (k + 1) * P, :])
        wot.append(t)

    theta = persist.tile([C8, N], F32R)
    phi = persist.tile([C8, M], F32R)
    g = [persist.tile([P, M], F32R, name=f"g{i}") for i in range(2)]
    gT = [persist.tile([P, C2], BF16, name=f"gT{mt}") for mt in range(MT)]

    ET = [[None] * MT for _ in range(NS)]
    L1 = [None] * NS
    ZT = [None] * NS
    RINV = [None] * NS

    def emit_score(i, mt):
        nsl = slice(i * SL, (i + 1) * SL)
        sp = spool.tile([P, SL], F32, name="sp", tag=f"s{(i * 3 + mt) % 4}", bufs=1)
        nc.tensor.matmul(
            sp,
            lhsT=phi[:, mt * P : (mt + 1) * P],
            rhs=theta[:, nsl],
            start=True,
            stop=True,
        )
        et = etp.tile([P, SL], BF16, name="et", tag=f"et{mt}")
        nc.scalar.activation(et, sp, ACTF.Exp)
        ET[i][mt] = et

    def emit_scores(i):
        for mt in range(MT):
            emit_score(i, mt)

    def emit_tree(i, fast=False):
        # tree-sum the 8 bf16 exp tiles so Z needs a single 128-contract
        # matmul. Levels split Pool/DVE (bf16 runs at 2x on DVE); levels 2+3
        # accumulate in place. The last DVE level (emit_l3) is emitted
        # separately so it never sits in front of an attend's reciprocal in
        # the in-order DVE queue.
        l1 = []
        for j in range(4):
            t = miscp.tile([P, SL], BF16, name="zl1", tag=f"zl1{j}", bufs=3)
            eng = nc.vector if (fast and j >= 2) or j == 3 else nc.gpsimd
            eng.tensor_add(t, ET[i][2 * j], ET[i][2 * j + 1])
            l1.append(t)
        nc.gpsimd.tensor_add(l1[0], l1[0], l1[1])
        nc.vector.tensor_add(l1[2], l1[2], l1[3])
        L1[i] = l1

    def emit_l3(i):
        l1 = L1[i]
        nc.vector.tensor_add(l1[0], l1[0], l1[2])
        ZT[i] = l1[0]

    # ---- phase 1: projections + pooling + g transposes -----------------
    with tc.tile_pool(name="ppsum", bufs=2, space="PSUM") as pp, tc.tile_pool(
        name="tpsum", bufs=1, space="PSUM"
    ) as tp:
        for ns in range(NS):
            nsl = slice(ns * SL, (ns + 1) * SL)
            msl = slice(ns * P, (ns + 1) * P)
            xr = [xfa[:, k, nsl] for k in range(4)]
            ps = [
                pp.tile(
                    [P, SL], F32, name="pp", tag=f"pp{mt}", bufs=1,
                )
                for mt in range(3)
            ]
            # last slice: k-major so only three matmuls wait on the final
            # x chunk (no successor slice to stall on the pool reads)
            if ns == NS - 1:
                order = [(mt, k) for k in range(4) for mt in (0, 1, 2)]
            else:
                order = [(mt, k) for mt in (1, 2, 0) for k in range(4)]
            for mt, k in order:
                nc.tensor.matmul(
                    ps[mt],
                    lhsT=wp[k][:, mt * P : (mt + 1) * P],
                    rhs=xr[k],
                    start=(k == 0),
                    stop=(k == 3),
                    skip_group_check=True,
                )

            # pooled phi written straight into partitions 0-63 (the DVE
            # access patterns cross partitions; no shift copy needed).
            # On the last slice phi + theta go first: they gate the final
            # scores and with them the whole phase-2 entry.
            def _pool_phi():
                nc.vector.tensor_reduce(
                    out=phi[:, msl],
                    in_=_pool_view(ps[0][C8:P, :]),
                    axis=AX.XY,
                    op=ALU.max,
                )

            def _pool_g():
                for i in range(2):
                    # last slice: one g pool on Pool so the pair finishes
                    # sooner (the transposes gate the first attend)
                    eng = nc.gpsimd if (ns == NS - 1 and i == 1) else nc.vector
                    eng.tensor_reduce(
                        out=g[i][:, msl],
                        in_=_pool_view(ps[1 + i]),
                        axis=AX.XY,
                        op=ALU.max,
                    )

            if ns == NS - 1:
                _pool_phi()
                # the (0,7) score reads theta SLICE 0 (long ready) — only
                # phi(7) gates it. theta(7) itself isn't needed until
                # scores(7) several iterations into phase 2, so its copy
                # drops to the back of the DVE queue.
                emit_score(0, ns)
                _pool_g()
                nc.vector.tensor_copy(out=theta[:, nsl], in_=ps[0][0:C8, :])
            else:
                _pool_g()
                _pool_phi()
                nc.scalar.copy(out=theta[:, nsl], in_=ps[0][0:C8, :])
            # transpose this slice's pooled g columns into gT[ns] (bf16 for
            # the attend matmuls)
            for i in range(2):
                t = tp.tile([P, P], F32R, name="tp", tag="tp")
                nc.tensor.transpose(t, g[i][:, msl], ident)
                nc.scalar.copy(out=gT[ns][:, i * P : (i + 1) * P], in_=t)
            # slice-0/1 scores ride in the phase-1 PE slack (x-DMA paced)
            if ns == 1:
                emit_score(0, 0)
            if 1 <= ns < NS - 1:
                emit_score(0, ns)
            if 2 <= ns < NS - 1:
                emit_score(1, ns - 2)
        # slice-0 tree, DVE-heavy: ready before its Z matmul a few us into
        # phase 2
        emit_tree(0, fast=True)
        emit_l3(0)

    # ---- phase 2: softmax / attend / project ---------------------------
    with tc.tile_pool(name="qpsum", bufs=2, space="PSUM") as qp:
        ATT = [None] * NS

        def emit_attend_ap(i, l3_next=False):
            # ct-major ap accumulation; the Z matmul + reciprocal slot in
            # after the ct=0 block (ct=1 for slice 0, whose tree only
            # finishes early in phase 2) so rinv is ready for the att
            # multiplies
            def _emit_z():
                # zp borrows an 'o' ring slot (free mid-attend), leaving a
                # psum bank for a fourth score tag
                zp = qp.tile([P, SL], F32, name="zp", tag="o")
                nc.tensor.matmul(
                    zp, lhsT=ones_b, rhs=ZT[i], start=True, stop=True,
                    skip_group_check=True,
                )
                rinv = miscp.tile([P, SL], F32, name="rinv", tag="rinv")
                nc.vector.reciprocal(rinv, zp)
                RINV[i] = rinv

            ap = [qp.tile([P, SL], F32, name="ap", tag="a", bufs=2) for _ in range(2)]
            for ct in range(2):
                for mt in range(MT):
                    nc.tensor.matmul(
                        ap[ct],
                        lhsT=gT[mt][:, ct * P : (ct + 1) * P],
                        rhs=ET[i][mt],
                        start=(mt == 0),
                        stop=(mt == MT - 1),
                        skip_group_check=True,
                    )
                    # slot the Z matmul mid-block so the reciprocal (and with
                    # it the att multiplies) starts a few matmuls earlier
                    if (ct, mt) == ((1, 3) if i == 0 else (0, 5)):
                        _emit_z()
            att = []
            for ct in range(2):
                t = miscp.tile([P, SL], BF16, name="att", tag=f"att{ct}", bufs=2)
                nc.vector.tensor_mul(t, ap[ct], RINV[i])
                att.append(t)
            ATT[i] = att
            # the next slice's final tree add right after the att multiplies:
            # ahead of the residual adds, so the mid-block Z never stalls
            if l3_next:
                emit_l3(i + 1)

        def emit_outproj(i, lo, w, dma_eng=None, tags=None):
            # out-projection + residual for columns [i*SL+lo, i*SL+lo+w);
            # one batched DMA writes all four 128-row DRAM blocks. Narrow
            # chunks group the ot accumulations into shared psum allocations
            # (fewer 'o' ring waits) and do one batched residual add per
            # group instead of four.
            att = ATT[i]
            hsl = slice(i * SL + lo, i * SL + lo + w)
            ob = miscp.tile([P, 4, w], BF16, name="ob", tag="ob", bufs=4)
            if w <= P:
                groups = [(0, 4)]
            elif w <= 2 * P:
                groups = [(0, 2), (2, 2)]
            else:
                groups = [(ot, 1) for ot in range(4)]
            final = i == NS - 1 and lo == SL // 2
            for gi, (base, cnt) in enumerate(groups):
                gtag = tags[gi] if tags else "o"
                pe_resid = (final or i == NS - 2) and gi == len(groups) - 1
                opg = qp.tile([P, cnt, w], F32, name="op", tag=gtag)
                for d in range(cnt):
                    for ct in range(2):
                        nc.tensor.matmul(
                            opg[:, d, :],
                            lhsT=wot[ct][:, (base + d) * P : (base + d + 1) * P],
                            rhs=att[ct][:, lo : lo + w],
                            start=(ct == 0),
                            stop=(ct == 1 and not pe_resid),
                            skip_group_check=True,
                        )
                    if pe_resid:
                        # very last group: accumulate the residual on the PE
                        # so the psum->SBUF move is a plain copy on the idle
                        # ACT engine instead of queueing in the DVE obs chain
                        nc.tensor.matmul(
                            opg[:, d, :],
                            lhsT=ident,
                            rhs=xfa[:, base + d, hsl],
                            start=False,
                            stop=True,
                            skip_group_check=True,
                        )
                if pe_resid:
                    nc.scalar.copy(out=ob[:, base : base + cnt, :], in_=opg)
                else:
                    nc.vector.tensor_add(
                        ob[:, base : base + cnt, :],
                        opg,
                        xfa[:, base : base + cnt, hsl].bitcast(F32),
                    )
            (dma_eng or nc.sync).dma_start(out=out_v[:, :, hsl], in_=ob)

        for mt in range(NS - 3, MT):
            emit_score(1, mt)
        emit_tree(1, fast=True)
        for i in range(NS):
            emit_attend_ap(i, l3_next=(i + 1 < NS))
            if i < NS - 1:
                chunks = [(0, SL)]
            else:
                chunks = [(0, SL // 2), (SL // 2, SL // 2)]
            ctags = [None, None]
            if i == NS - 1:
                # the attmuls have released the 'a' psum ring by now;
                # alternating 'o'/'a' removes every endgame ring wait
                ctags = [("o", "a"), ("o", "a")]
            for ci, (lo, w) in enumerate(chunks):
                emit_outproj(
                    i, lo, w,
                    dma_eng=None,
                    tags=ctags[ci],
                )
            if i + 2 < NS:
                emit_scores(i + 2)
                emit_tree(i + 2)


def build_nc():
    nc = bass.Bass(target_bir_lowering=False, trn_type="TRN2")
    with tile.TileContext(nc) as tc:
        with ExitStack() as ctx:
            emit(nc, tc, ctx)
    bass_rust.generate_event_semaphores(nc)
    return nc


def kernel(x, w_theta, w_phi, w_g, w_o, gamma):
    x = np.asarray(x, dtype=np.float32)
    B = x.shape[0]
    wproj = np.ascontiguousarray(
        np.concatenate(
            [np.asarray(w_theta).T, np.asarray(w_phi).T, np.asarray(w_g).T], axis=1
        ),
        dtype=np.float32,
    ).astype(ml_dtypes.bfloat16)
    wo_t = np.ascontiguousarray(
        (np.float32(gamma) * np.asarray(w_o)).T, dtype=np.float32
    ).astype(ml_dtypes.bfloat16)

    nc = build_nc()
    in_maps = []
    for b in range(B):
        xb = np.ascontiguousarray(x[b].reshape(C, N))
        in_maps.append({"x": xb, "wproj": wproj, "wo": wo_t})
    # retry: rare transient NRT_EXEC_UNIT_UNRECOVERABLE from stale device
    # state clears on re-execution
    last_err = None
    for attempt in range(3):
        try:
            res = run_bass_kernel_spmd(nc, in_maps, core_ids=list(range(B)))
            break
        except Exception as e:  # noqa: BLE001
            last_err = e
            time.sleep(2.0)
    else:
        raise last_err
    out = np.stack(
        [res.results[b]["out"].reshape(C, 64, 64) for b in range(B)]
    ).astype(np.float32)
    return out
